# revision 32
# baseline (speedup 1.0000x reference)
"""DMN4 few-shot episode loss (retrieval_knn) on 8 TRN2 NeuronCores.

Data-parallel sharding: core i handles batch i//2, query half i%2 (38 queries,
batch queries padded 75->76).  Per core, on device:
  - cosine similarity of 3800 query descriptors vs 2500 support descriptors
    (fp32r matmuls, support pre-scaled by its inverse L2 norms),
  - per-class max, global argmax (equality + descending-iota max-reduce),
    top2-class-margin per descriptor,
  - DMNN mask: within each query's 100 descriptors, keep a descriptor iff it
    has the max margin among descriptors sharing its nearest support index,
  - query_value -> logits -> log-softmax -> per-query NLL.
Host gathers the 8x[38] per-query losses and means the 300 real ones.
"""

import os
import numpy as np

# ---- problem constants (hardcoded; kernel.py must be self-contained) ----
B, Q, C, HW = 4, 75, 640, 100
NW, KS = 5, 5                    # n_way, k_shot
S = NW * KS                      # 25 support images
MS, MSP = KS * HW, 512           # 500 support desc per class; 512 = psum bank stride
KC = C // 128                    # 5 contraction chunks
NQ = 38                          # queries per core (75 padded to 76, split in 2)
QD, QDP = NQ * HW, 3840          # 3800 query descriptors, padded to 30 tiles of 128
NT = QDP // 128                  # 30 M-tiles
ITEMP = 5.0                      # 1 / temperature(0.2)
EPS = 1e-12
BIG = 1e6

_CACHE = {}


def _segments(t):
    """Query-DMA segments for M-tile t: list of (m_off, query_idx, h0, length).
    query_idx == NQ means padding (memset instead of DMA)."""
    segs = []
    m = t * 128
    end = m + 128
    while m < end:
        qi, h0 = m // HW, m % HW
        ln = min(end - m, HW - h0)
        if qi >= NQ:
            segs.append((m - t * 128, NQ, 0, end - m))
            break
        segs.append((m - t * 128, qi, h0, ln))
        m += ln
    return segs


def _gsegments(g, width=640):
    """Query-aligned segments of qdesc range [g*width, (g+1)*width)."""
    segs = []
    m = g * width
    end = m + width
    while m < end:
        qi, h0 = m // HW, m % HW
        ln = min(end - m, HW - h0)
        if qi >= NQ:
            segs.append((m - g * width, NQ, 0, end - m))
            break
        segs.append((m - g * width, qi, h0, ln))
        m += ln
    return segs


def _build_nc():
    from concourse import bass, mybir
    from concourse.tile import TileContext
    from contextlib import ExitStack

    f32 = mybir.dt.float32
    f32r = mybir.dt.float32r
    bf16 = mybir.dt.bfloat16
    i32 = mybir.dt.int32
    AX = mybir.AxisListType
    OP = mybir.AluOpType
    ACT = mybir.ActivationFunctionType

    nc = bass.Bass()
    sup = nc.declare_dram_parameter("support", [S, C, HW], f32, isOutput=False)
    qry = nc.declare_dram_parameter("query", [NQ, C, HW], f32, isOutput=False)
    lab = nc.declare_dram_parameter("labels", [NQ, 1], i32, isOutput=False)
    out_ext = nc.declare_dram_parameter("out", [NQ, 1], f32, isOutput=True)

    with ExitStack() as ctx:
        tc = ctx.enter_context(TileContext(nc))
        dram = ctx.enter_context(tc.tile_pool(name="dram", bufs=1, space="DRAM"))
        stats_d = dram.tile([QDP, 7], f32)        # cols: 0 enc, 1 diff_raw, 2:7 cmax
        qssq_d = dram.tile([QDP, 1], f32)

        consts = ctx.enter_context(tc.tile_pool(name="consts", bufs=1))
        spool = ctx.enter_context(tc.tile_pool(name="s", bufs=1))
        ppool = ctx.enter_context(tc.tile_pool(name="ps", bufs=6, space="PSUM"))
        qspsum = ctx.enter_context(tc.tile_pool(name="psq", bufs=2, space="PSUM"))

        ones_col = consts.tile([128, 1], f32)
        ones_row = consts.tile([1, 128], f32)
        ones_cbf = consts.tile([128, 1], bf16)
        iota_t = consts.tile([128, NW * MS], f32)
        # DVE memset cannot write f32r on this toolchain; ACT Copy with
        # scale=0 bias=c writes constants through a rounded f32r output.
        nc.scalar.activation(ones_col[:].bitcast(f32r), ones_col[:], ACT.Copy,
                             bias=1.0, scale=0.0)
        nc.scalar.activation(ones_row[:].bitcast(f32r), ones_row[:], ACT.Copy,
                             bias=1.0, scale=0.0)
        nc.vector.memset(ones_cbf[:], 1.0)
        nc.gpsimd.iota(iota_t[:], pattern=[[-1, NW * MS]], base=NW * MS - 1,
                       channel_multiplier=0, allow_small_or_imprecise_dtypes=True)

        # ---------------- prologue: load S, fold 1/||s|| into S ----------------
        s_tiles = [spool.tile([128, NW, MSP], f32, tag=f"s{k}", name=f"s{k}") for k in range(KC)]
        for k in range(KC):
            for n in range(NW):
                eng = nc.scalar if (k * NW + n) % 2 == 0 else nc.sync
                eng.dma_start(
                    out=s_tiles[k][:, n, 0:MS].rearrange("p (s h) -> p s h", s=KS)
                        .bitcast(f32r),
                    in_=sup[KS * n:KS * n + KS, 128 * k:128 * k + 128, :]
                        .rearrange("s c h -> c s h").bitcast(f32r))

        ssq_ps = [ppool.tile([1, MSP], f32, tag="simi", name="ssqps") for _ in range(NW)]
        work_cm = tc.tile_pool(name="work", bufs=2)
        work = work_cm.__enter__()
        for k in range(KC):
            sq = work.tile([128, NW, MSP], f32, tag="ssq")
            nc.scalar.activation(sq[:, :, 0:MS].bitcast(f32r), s_tiles[k][:, :, 0:MS], ACT.Square)
            for n in range(NW):
                nc.tensor.matmul(ssq_ps[n][0:1, 0:MS],
                                 lhsT=ones_col[:].bitcast(f32r),
                                 rhs=sq[:, n, 0:MS].bitcast(f32r),
                                 start=(k == 0), stop=(k == KC - 1))

        norms = work.tile([1, NW, MSP], f32, tag="norms")
        sinvr = consts.tile([1, NW, MSP], f32)
        bc_ps = [ppool.tile([128, MSP], f32, tag="simi", name="bcps") for _ in range(NW)]
        sinvb = consts.tile([128, NW, MSP], f32)
        for n in range(NW):
            nc.scalar.activation(norms[0:1, n, 0:MS], ssq_ps[n][0:1, 0:MS], ACT.Sqrt)
            with nc.allow_low_precision(reason="fp32r rounding of 1/||s||"):
                nc.vector.reciprocal(sinvr[0:1, n, 0:MS].bitcast(f32r),
                                     norms[0:1, n, 0:MS])
            nc.tensor.matmul(bc_ps[n][:, 0:MS],
                             lhsT=ones_row[:].bitcast(f32r),
                             rhs=sinvr[0:1, n, 0:MS].bitcast(f32r),
                             start=True, stop=True)
            nc.scalar.copy(sinvb[:, n, 0:MS], bc_ps[n][:, 0:MS])
        for k in range(KC):
            for n in range(NW):
                eng = nc.vector if k < 3 else nc.gpsimd
                eng.tensor_tensor(out=s_tiles[k][:, n, 0:MS].bitcast(f32r),
                                  in0=s_tiles[k][:, n, 0:MS],
                                  in1=sinvb[:, n, 0:MS], op=OP.mult)
        work_cm.__exit__(None, None, None)

        # ---------------- main loop: simi matmuls + per-descriptor stats ----------------
        qgpool = ctx.enter_context(tc.tile_pool(name="qg", bufs=3))
        simi_pool = ctx.enter_context(tc.tile_pool(name="simisb", bufs=2))
        eq_pool = ctx.enter_context(tc.tile_pool(name="eqp", bufs=1))
        st_pool = ctx.enter_context(tc.tile_pool(name="stp", bufs=4))

        bpool = ctx.enter_context(tc.tile_pool(name="b", bufs=1))
        g_dup = bpool.tile([2 * NQ, HW], f32, tag="gd", name="gd")
        dr_dup = bpool.tile([2 * NQ, HW], f32, tag="dd", name="dd")
        ss_dup = bpool.tile([2 * NQ, HW], f32, tag="sd", name="sd")
        cm_sb = bpool.tile([NQ, HW, NW], f32, tag="cm", name="cm")
        lab_sb = bpool.tile([NQ, 1], i32, tag="lab", name="lab")
        nc.sync.dma_start(out=lab_sb[:], in_=lab[:, :])

        def _bchunk(q0, q1, g):
            nq = q1 - q0
            colg = stats_d[q0 * HW:q1 * HW, 0:1].rearrange("(q m) c -> q (m c)", q=nq)
            cold = stats_d[q0 * HW:q1 * HW, 1:2].rearrange("(q m) c -> q (m c)", q=nq)
            colss = qssq_d[q0 * HW:q1 * HW, 0:1].rearrange("(q m) c -> q (m c)", q=nq)
            for h in range(2):
                sl = slice(h * NQ + q0, h * NQ + q1)
                nc.gpsimd.dma_start(out=g_dup[sl, :], in_=colg)
                nc.scalar.dma_start(out=dr_dup[sl, :], in_=cold)
                nc.gpsimd.dma_start(out=ss_dup[sl, :], in_=colss)
            eng = nc.sync if g % 2 == 0 else nc.scalar
            eng.dma_start(out=cm_sb[q0:q1], in_=stats_d[q0 * HW:q1 * HW, 2:7]
                          .rearrange("(q m) c -> q m c", q=nq))

        GW = 640                     # 5 M-tiles per Q load group
        qdone = 0
        for g in range(NT // 5):
            qg = [qgpool.tile([128, GW], f32, tag=f"qg{k}", name=f"qg{k}")
                  for k in range(KC)]
            for si, (m_off, qi, h0, ln) in enumerate(_gsegments(g, GW)):
                for k in range(KC):
                    if qi >= NQ:
                        nc.scalar.activation(
                            qg[k][:, m_off:m_off + ln].bitcast(f32r),
                            qg[k][:, m_off:m_off + ln], ACT.Copy,
                            bias=0.0, scale=0.0)
                    else:
                        eng = nc.sync if (si * KC + k) % 2 == 0 else nc.scalar
                        eng.dma_start(
                            out=qg[k][:, m_off:m_off + ln].bitcast(f32r),
                            in_=qry[qi, 128 * k:128 * k + 128, h0:h0 + ln]
                                .bitcast(f32r))

            for tt in range(5):
                t = 5 * g + tt
                msl = slice(tt * 128, (tt + 1) * 128)
                ps = [ppool.tile([128, MSP], f32, tag="simi", name="bcps")
                      for _ in range(NW)]
                for k in range(KC):
                    for n in range(NW):
                        nc.tensor.matmul(ps[n][:, 0:MS],
                                         lhsT=qg[k][:, msl].bitcast(f32r),
                                         rhs=s_tiles[k][:, n, 0:MS].bitcast(f32r),
                                         start=(k == 0), stop=(k == KC - 1))

                # query sum-of-squares (bf16 squares: rel err ~2e-4)
                qs_ps = qspsum.tile([1, 128], f32, tag="qs")
                for k in range(KC):
                    qsq = st_pool.tile([128, 128], bf16, tag="qsq")
                    nc.gpsimd.tensor_tensor(out=qsq[:], in0=qg[k][:, msl],
                                            in1=qg[k][:, msl], op=OP.mult)
                    nc.tensor.matmul(qs_ps[0:1, :], lhsT=ones_cbf[:], rhs=qsq[:],
                                     start=(k == 0), stop=(k == KC - 1))
                qs_sb = st_pool.tile([1, 128], f32, tag="qs_sb")
                nc.scalar.copy(qs_sb[:], qs_ps[0:1, :])
                nc.gpsimd.dma_start(out=qssq_d[128 * t:128 * t + 128, 0:1]
                                    .rearrange("m c -> c m"),
                                    in_=qs_sb[:])

                simi_sb = simi_pool.tile([128, NW * MS], f32, tag="simi_sb")
                sb3 = simi_sb[:].rearrange("p (n s) -> p n s", n=NW)
                for n in range(NW):
                    nc.scalar.copy(sb3[:, n, :], ps[n][:, 0:MS])

                st = st_pool.tile([128, 8], f32, tag="st")
                gmax = st_pool.tile([128, 1], f32, tag="gmax")
                m5 = st_pool.tile([128, 8], f32, tag="m5")
                top2 = st_pool.tile([128, 1], f32, tag="top2")
                nc.vector.reduce_max(st[:, 2:7], sb3, axis=AX.X)
                nc.vector.reduce_max(gmax[:], st[:, 2:7], axis=AX.X)
                # enc = sum_j (simi==gmax)*iotaD (argmax unique a.s.)
                eq_t = eq_pool.tile([128, NW * MS], f32, tag="eq")
                nc.vector.scalar_tensor_tensor(out=eq_t[:], in0=simi_sb[:],
                                               scalar=gmax[:], in1=iota_t[:],
                                               op0=OP.is_equal, op1=OP.mult,
                                               accum_out=st[:, 0:1])
                nc.vector.tensor_scalar(out=m5[:, 0:5], in0=st[:, 2:7],
                                        scalar1=gmax[:], scalar2=None,
                                        op0=OP.is_equal)
                nc.vector.scalar_tensor_tensor(out=m5[:, 0:5], in0=m5[:, 0:5],
                                               scalar=-BIG, in1=st[:, 2:7],
                                               op0=OP.mult, op1=OP.add)
                nc.vector.reduce_max(top2[:], m5[:, 0:5], axis=AX.X)
                nc.vector.tensor_tensor(out=st[:, 1:2], in0=gmax[:], in1=top2[:],
                                        op=OP.subtract)
                nc.gpsimd.dma_start(out=stats_d[128 * t:128 * t + 128, :],
                                    in_=st[:, 0:7])

            qnow = min(NQ, (GW * (g + 1)) // HW)
            if qnow > qdone:
                _bchunk(qdone, qnow, g)
                qdone = qnow

        # ---------------- phase B: DMNN mask + loss, query-major layout ----------------
        qinv = bpool.tile([2 * NQ, HW], f32, tag="qinv")
        nc.scalar.activation(qinv[:], ss_dup[:], ACT.Sqrt)
        nc.vector.tensor_scalar_max(qinv[:], qinv[:], EPS)
        nc.vector.reciprocal(qinv[:], qinv[:])
        d_dup = bpool.tile([2 * NQ, HW], f32, tag="ddup")
        nc.vector.tensor_tensor(out=d_dup[:], in0=dr_dup[:], in1=qinv[:], op=OP.mult)

        # half layouts: row (q + 38h) holds columns [50h, 50h+50) of query q
        HH = HW // 2
        g_half = bpool.tile([2 * NQ, HH], f32, tag="gh")
        d_half = bpool.tile([2 * NQ, HH], f32, tag="dh")
        for h in range(2):
            sl = slice(h * NQ, (h + 1) * NQ)
            nc.sync.dma_start(out=g_half[sl, :], in_=g_dup[sl, h * HH:(h + 1) * HH])
            nc.sync.dma_start(out=d_half[sl, :], in_=d_dup[sl, h * HH:(h + 1) * HH])

        grp_h = bpool.tile([2 * NQ, HW], f32, tag="grph")
        for mb in range(2):
            msl = slice(mb * HH, (mb + 1) * HH)
            eq3 = bpool.tile([2 * NQ, HH, HH], f32, tag="eq3", name="eq3")
            nc.vector.tensor_tensor(
                out=eq3[:],
                in0=g_dup[:, msl].rearrange("p (m o) -> p m o", o=1)
                    .broadcast_to((2 * NQ, HH, HH)),
                in1=g_half[:].rearrange("p (o j) -> p o j", o=1)
                    .broadcast_to((2 * NQ, HH, HH)),
                op=OP.is_equal)
            mk3 = bpool.tile([2 * NQ, HH, HH], f32, tag="mk3", name="mk3")
            nc.vector.tensor_tensor(
                out=mk3[:], in0=eq3[:],
                in1=d_half[:].rearrange("p (o j) -> p o j", o=1)
                    .broadcast_to((2 * NQ, HH, HH)),
                op=OP.mult)
            nc.vector.reduce_max(grp_h[:, msl], mk3[:], axis=AX.X)

        grp_hi = bpool.tile([NQ, HW], f32, tag="grphi")
        nc.sync.dma_start(out=grp_hi[:], in_=grp_h[NQ:2 * NQ, :])
        grp = bpool.tile([NQ, HW], f32, tag="grp")
        nc.vector.tensor_tensor(out=grp[:], in0=grp_h[0:NQ, :], in1=grp_hi[:],
                                op=OP.max)
        keep = bpool.tile([NQ, HW], f32, tag="keep")
        nc.vector.tensor_tensor(out=keep[:], in0=d_dup[0:NQ, :], in1=grp[:],
                                op=OP.is_ge)
        w = bpool.tile([NQ, HW], f32, tag="w")
        nc.vector.tensor_tensor(out=w[:], in0=keep[:], in1=qinv[0:NQ, :], op=OP.mult)

        qv = bpool.tile([NQ, NW], f32, tag="qv")
        sc100 = bpool.tile([NQ, HW], f32, tag="sc100")
        for n in range(NW):
            nc.vector.scalar_tensor_tensor(out=sc100[:], in0=cm_sb[:, :, n],
                                           scalar=ITEMP, in1=w[:],
                                           op0=OP.mult, op1=OP.mult,
                                           accum_out=qv[:, n:n + 1])

        lmax = bpool.tile([NQ, 1], f32, tag="lmax")
        xs = bpool.tile([NQ, NW], f32, tag="xs")
        ex = bpool.tile([NQ, NW], f32, tag="ex")
        sumexp = bpool.tile([NQ, 1], f32, tag="sumexp")
        logz = bpool.tile([NQ, 1], f32, tag="logz")
        nc.vector.reduce_max(lmax[:], qv[:], axis=AX.X)
        nc.vector.tensor_scalar(out=xs[:], in0=qv[:], scalar1=lmax[:], scalar2=None,
                                op0=OP.subtract)
        nc.scalar.activation(ex[:], xs[:], ACT.Exp, accum_out=sumexp[:])
        nc.scalar.activation(logz[:], sumexp[:], ACT.Ln)

        iot5 = bpool.tile([NQ, NW], i32, tag="iot5")
        nc.gpsimd.iota(iot5[:], pattern=[[1, NW]], base=0, channel_multiplier=0)
        labf = bpool.tile([NQ, 1], f32, tag="labf")
        nc.vector.tensor_copy(out=labf[:], in_=lab_sb[:])
        iot5f = bpool.tile([NQ, NW], f32, tag="iot5f")
        nc.vector.tensor_copy(out=iot5f[:], in_=iot5[:])
        onehot = bpool.tile([NQ, NW], f32, tag="onehot")
        nc.vector.tensor_scalar(out=onehot[:], in0=iot5f[:], scalar1=labf[:],
                                scalar2=None, op0=OP.is_equal)
        sc5 = bpool.tile([NQ, NW], f32, tag="sc5")
        picked = bpool.tile([NQ, 1], f32, tag="picked")
        nc.vector.scalar_tensor_tensor(out=sc5[:], in0=xs[:], scalar=1.0,
                                       in1=onehot[:], op0=OP.mult, op1=OP.mult,
                                       accum_out=picked[:])
        lossq = bpool.tile([NQ, 1], f32, tag="lossq")
        nc.vector.tensor_tensor(out=lossq[:], in0=logz[:], in1=picked[:],
                                op=OP.subtract)
        nc.sync.dma_start(out=out_ext[:, :], in_=lossq[:])

    return nc


def _legalize_waits(nc):
    """This toolchain's walrus accepts at most one sync-wait per instruction
    (two on EventSemaphore).  Tile emits more on join points (matmuls after
    multiple DMAs, the kernel-tail drain).  Spill the excess onto same-engine
    NoOps placed just before the instruction -- semantically identical, since
    an engine executes its instruction stream (and their waits) in order."""
    from concourse import mybir
    n = 0
    for f in nc.m.functions:
        for bb in f.blocks:
            out = []
            for inst in bb.instructions:
                si = inst.sync_info
                if si is not None and si.on_wait:
                    cap = 2 if isinstance(inst, mybir.InstEventSemaphore) else 1
                    waits = list(si.on_wait)
                    if len(waits) > cap:
                        for w in waits[:-cap]:
                            nop = mybir.InstNoOp(
                                name=f"wspill-{n}", engine=inst.engine,
                                ins=[], outs=[])
                            nop.sync_info = mybir.SyncInfo(on_wait=[w],
                                                           on_update=[])
                            out.append(nop)
                            n += 1
                        si.on_wait = waits[-cap:]
                out.append(inst)
            bb.instructions[:] = out
    return n


def _enable_ldw_opt():
    """Each of our 25 simi matmuls per tile reloads the same stationary
    Q-tile 5x; walrus's ldw-opt pass dedups consecutive identical weight
    loads but concourse pins it off.  Flip the flag on the walrus cmdline."""
    if _CACHE.get("ldw_patched"):
        return
    import concourse.bass_utils as bu
    orig = bu.run_command

    def run_command_ldw(cmd, *a, **k):
        cmd = [c.replace("--enable-ldw-opt=false", "--enable-ldw-opt=true")
               if isinstance(c, str) else c for c in cmd]
        return orig(cmd, *a, **k)

    bu.run_command = run_command_ldw
    _CACHE["ldw_patched"] = True


def _get_nc():
    if "nc" not in _CACHE:
        nc = _build_nc()
        _legalize_waits(nc)
        _CACHE["nc"] = nc
    return _CACHE["nc"]


def _make_in_maps(support_xf, query_xf, query_y):
    sup = np.ascontiguousarray(
        np.asarray(support_xf, dtype=np.float32).reshape(B, S, C, HW))
    qry = np.asarray(query_xf, dtype=np.float32).reshape(B, Q, C, HW)
    qy = np.asarray(query_y).astype(np.int32).reshape(B, Q)
    in_maps = []
    for core in range(8):
        bi, half = core // 2, core % 2
        q0 = half * NQ
        qn = min(NQ, Q - q0)
        qshard = np.zeros((NQ, C, HW), np.float32)
        qshard[:qn] = qry[bi, q0:q0 + qn]
        labs = np.zeros((NQ, 1), np.int32)
        labs[:qn, 0] = qy[bi, q0:q0 + qn]
        in_maps.append({
            "support": np.ascontiguousarray(sup[bi]),
            "query": qshard,
            "labels": labs,
        })
    return in_maps


def _run(inputs, trace=False, **kw):
    from concourse.bass_utils import run_bass_kernel_spmd
    nc = _get_nc()
    in_maps = _make_in_maps(inputs["support_xf"], inputs["query_xf"],
                            inputs["query_y"])
    res = run_bass_kernel_spmd(nc, in_maps, core_ids=list(range(8)),
                               trace=trace, **kw)
    losses = []
    for core in range(8):
        half = core % 2
        qn = min(NQ, Q - half * NQ)
        losses.append(np.asarray(res.results[core]["out"])[:qn, 0])
    loss = np.float32(np.mean(np.concatenate(losses)))
    return loss, res


def kernel(**inputs) -> np.ndarray:
    loss, _ = _run(inputs, trace=False)
    return np.asarray(loss, dtype=np.float32)


if __name__ == "__main__":
    nc = _build_nc()
    print("built ok; instructions:",
          sum(len(bb.bb.instructions) for bb in nc.bb_map.values()))
